# revision 8
# baseline (speedup 1.0000x reference)
"""GroupedMLP (MoE, uniform routing) on 8 NeuronCores via expert parallelism.

v4 over v3 (1346.9 us):
  - MM2 runs fo-blocks 28..31 (4 of 32, 12.5% of its contraction) as fp8
    e4m3 DoubleRow matmuls: 2 DR instructions replace 4 bf16 ones per
    chain.  Scale split keeps the psum at scale 1: host packs
    w_down[...,28:32]*4 as e4m3 and w_gate_up up-half cols 28:32 /4, so
    the DVE writes hid/4 to e4m3 with no extra ops.  Predicted end-to-end
    rel err 1.39e-2 (sim, real inputs) vs the 2e-2 gate.
  - Startup: first wgu tile ships as 3 slices (gate ho0-3 / gate ho4-15 /
    up) interleaved with 2 half-size x chunks so the first psum chain
    starts ~4 us earlier; NWARM 22 -> 6 (HAM needs ~3.4 us busy), memset
    on GpSimd (passes the preamble barrier earlier than DVE).
  - Tail: the very last chain is split into two N=256 chains with their
    own copy+store so the final transfer is 128 KB instead of 256 KB.

Math (per expert e, tokens pre-sorted):
    gate_up = x_e @ w_gate_up[e].T        # [2048, 8192]
    hidden  = silu(gate) * up             # [2048, 4096]
    out_e   = hidden @ w_down[e].T        # [2048, 2048]

Core i handles expert i (T/E = 2048 tokens).  Everything is computed in the
transposed domain so both matmul operands have the contraction dim on the
SBUF partition axis with zero on-chip transposes:
    gate_upT[f, t] = sum_h w_guT[h, f] * xT[h, t]
    outT[h', t]    = sum_f w_dnT[f, h'] * hidT[f, t]
Host packs x/w into bf16/e4m3 tiles laid out exactly as SBUF wants them;
PSUM accumulates in fp32.
"""

import numpy as np
import ml_dtypes

E, H, F, T = 8, 2048, 4096, 16384
TPE = T // E          # 2048 tokens per expert/core
NCORES = 8
TB = 512              # token block (PSUM bank width in fp32)
NTB = TPE // TB       # 4
NHO = H // 128        # 16 contraction blocks for MM1
NFO = F // 128        # 32 contraction blocks for MM2
NDR = 6               # fo-blocks of MM2 computed in fp8 DoubleRow
NPAIR = NDR // 2      # DoubleRow instructions per MM2 chain
NFO_BF = NFO - NDR    # 26 bf16 fo-blocks
NHB = H // 128        # 16 output h'-blocks
NPASS = 2             # token-half passes
TBP = NTB // NPASS    # 2 token blocks per pass
NWARM = 13            # PE warm-up matmuls during startup DMA
# MM2 chain order: bf16 blocks with the 3 DoubleRow MMs interleaved near the
# end, one bf16 between consecutive DRs so each DR's 256-col LDWEIGHTS hides
# under a bf16 matmul (DR->DR back-to-back stalls ~400 ns on LDW).
MM2_SEQ = (
    [("bf", fo) for fo in range(NFO_BF - NPAIR)]
    + [x for j in range(NPAIR) for x in (("dr", j), ("bf", NFO_BF - NPAIR + j))]
)
DRS = 4.0             # fp8 scale split: hid/DRS in e4m3, w_down*DRS in e4m3

BF16 = ml_dtypes.bfloat16
F8 = ml_dtypes.float8_e4m3   # TRN FP8_EXP4 bit-compatible

_CACHE = {}


def _split_multiwaits(nc, mybir, bass_rust):
    """walrus CTRL-format instructions on this compiler accept only one sem
    wait; hoist extra waits onto single-wait NOPs spliced just before."""
    for f in nc.m.functions:
        for b in f.blocks:
            new_insts = []
            for inst in b.instructions:
                si = inst.sync_info
                if si is not None and si.on_wait and len(si.on_wait) > 1:
                    waits = list(si.on_wait)
                    for w in waits[:-1]:
                        nop = mybir.InstNoOp(
                            name=f"I-waitsplit-{nc.next_id()}", ins=[], outs=[]
                        )
                        nop.engine = inst.engine
                        nop.sync_info = bass_rust.SyncInfo(on_wait=[w], on_update=[])
                        new_insts.append(nop)
                    si.on_wait = [waits[-1]]
                new_insts.append(inst)
            b.instructions[:] = new_insts
    return nc


def _build():
    import concourse.bass as bass
    import concourse.mybir as mybir
    import concourse.tile as tile
    import bass_rust
    from concourse.bass import ts

    nc = bass.Bass("TRN2", target_bir_lowering=False, debug=False)
    xt = nc.dram_tensor(
        "xt", [128, NTB, NHO, TB], mybir.dt.bfloat16, kind="ExternalInput"
    )
    wgu = nc.dram_tensor(
        "wgu", [NFO, 128, 2, NHO, 128], mybir.dt.bfloat16, kind="ExternalInput"
    )
    wdn = nc.dram_tensor(
        "wdn", [NHB, 128, NFO_BF, 128], mybir.dt.bfloat16, kind="ExternalInput"
    )
    wdn8 = nc.dram_tensor(
        "wdn8", [NHB, 128, NPAIR, 2, 128], mybir.dt.float8e4, kind="ExternalInput"
    )
    outT = nc.dram_tensor("outT", [H, TPE], mybir.dt.float32, kind="ExternalOutput")

    with tile.TileContext(nc) as tc:
        with (
            tc.tile_pool(name="xtp", bufs=1) as xt_pool,
            tc.tile_pool(name="wrm", bufs=1) as wrm_pool,
            tc.tile_pool(name="wgup", bufs=3) as wgu_pool,
            tc.tile_pool(name="wdnp", bufs=3) as wdn_pool,
            tc.tile_pool(name="wdn8p", bufs=3) as wdn8_pool,
            tc.tile_pool(name="hidp", bufs=1) as hid_pool,
            tc.tile_pool(name="hid8p", bufs=1) as hid8_pool,
            tc.tile_pool(name="tmpp", bufs=2) as tmp_pool,
            tc.tile_pool(name="obp", bufs=3) as ob_pool,
            tc.tile_pool(name="pgu", bufs=6, space="PSUM") as pgu_pool,
            tc.tile_pool(name="po", bufs=2, space="PSUM") as po_pool,
        ):
            # --- PE warm-up: flip the HAM clock gate to 2.4 GHz while the
            # first weight/x DMAs are in flight (~3.6 us of cold matmuls).
            wtile = wrm_pool.tile([128, TB], mybir.dt.bfloat16)
            nc.gpsimd.memset(wtile[:], 0)
            pw = po_pool.tile([128, TB], mybir.dt.float32, tag="po")
            for k in range(NWARM):
                nc.tensor.matmul(
                    pw[:],
                    wtile[:, 0:128],
                    wtile[:],
                    start=(k == 0),
                    stop=(k == NWARM - 1),
                )

            # jp=0 weights + x token-block 0, in fine slices so the first
            # psum chain starts as soon as ~0.4 MB has landed.
            wp0 = wgu_pool.tile([128, 2, NHO, 128], mybir.dt.bfloat16, tag="wgu")
            xt_t = xt_pool.tile([128, NTB, NHO, TB], mybir.dt.bfloat16)
            nc.sync.dma_start(wp0[:, 0, 0:4], wgu[0][:, 0, 0:4])
            nc.sync.dma_start(xt_t[:, 0, 0:2], xt[:, 0, 0:2])
            nc.sync.dma_start(xt_t[:, 0, 2:4], xt[:, 0, 2:4])
            nc.sync.dma_start(wp0[:, 0, 4:], wgu[0][:, 0, 4:])
            nc.sync.dma_start(xt_t[:, 0, 4:10], xt[:, 0, 4:10])
            nc.sync.dma_start(wp0[:, 1], wgu[0][:, 1])
            nc.sync.dma_start(xt_t[:, 0, 10:], xt[:, 0, 10:])
            h = NHO // 2
            for tb in range(1, TBP):
                nc.sync.dma_start(xt_t[:, tb, 0:h], xt[:, tb, 0:h])
                nc.sync.dma_start(xt_t[:, tb, h:], xt[:, tb, h:])

            for p in range(NPASS):
                hid = hid_pool.tile(
                    [128, NFO_BF, TBP * TB], mybir.dt.bfloat16, tag="hid"
                )
                hid8 = hid8_pool.tile(
                    [128, NPAIR, 2, TBP * TB], mybir.dt.float8e4, tag="hid8"
                )
                # ---- MM1: hid = silu(gate) * up for this token half ----
                for jp in range(NFO):  # paired gate-(jp) / up-(jp) blocks
                    if p == 0 and jp == 0:
                        wp = wp0
                    else:
                        wp = wgu_pool.tile(
                            [128, 2, NHO, 128], mybir.dt.bfloat16, tag="wgu"
                        )
                        nc.sync.dma_start(wp[:], wgu[jp])
                    for tbi in range(TBP):
                        tb = p * TBP + tbi
                        pg = pgu_pool.tile([128, TB], mybir.dt.float32, tag="pgu")
                        pu = pgu_pool.tile([128, TB], mybir.dt.float32, tag="pgu")
                        for ho in range(NHO):
                            nc.tensor.matmul(
                                pg[:],
                                wp[:, 0, ho, :],
                                xt_t[:, tb, ho, :],
                                start=(ho == 0),
                                stop=(ho == NHO - 1),
                            )
                        for ho in range(NHO):
                            nc.tensor.matmul(
                                pu[:],
                                wp[:, 1, ho, :],
                                xt_t[:, tb, ho, :],
                                start=(ho == 0),
                                stop=(ho == NHO - 1),
                            )
                        tmp = tmp_pool.tile([128, TB], mybir.dt.float32, tag="tmp")
                        nc.scalar.activation(
                            tmp[:], pg[:], mybir.ActivationFunctionType.Silu
                        )
                        if jp < NFO_BF:
                            nc.vector.tensor_mul(
                                hid[:, jp, ts(tbi, TB)], tmp[:], pu[:]
                            )
                        else:
                            j, i2 = divmod(jp - NFO_BF, 2)
                            # up-half weights for these jp are pre-scaled by
                            # 1/DRS on host, so this lands hid/DRS in e4m3.
                            nc.vector.tensor_mul(
                                hid8[:, j, i2, ts(tbi, TB)], tmp[:], pu[:]
                            )

                # ---- MM2: out = hid @ w_down.T for this token half ----
                if p == 0:
                    # pass-1 x chunks, now that the pass-0 weight stream is
                    # well ahead of the PE.
                    for tb in range(TBP, NTB):
                        nc.sync.dma_start(xt_t[:, tb], xt[:, tb])
                for hb in range(NHB):
                    wd = wdn_pool.tile(
                        [128, NFO_BF, 128], mybir.dt.bfloat16, tag="wdn"
                    )
                    nc.sync.dma_start(wd[:], wdn[hb])
                    wd8 = wdn8_pool.tile(
                        [128, NPAIR, 2, 128], mybir.dt.float8e4, tag="wdn8"
                    )
                    nc.sync.dma_start(wd8[:], wdn8[hb])
                    ob = ob_pool.tile([128, TBP * TB], mybir.dt.float32, tag="ob")
                    last = p == NPASS - 1 and hb == NHB - 1
                    for tbi in range(TBP):
                        if last and tbi == TBP - 1:
                            # final chain split 256/128/128 so the very last
                            # transfer is only 64 KB
                            for off, w in ((0, 256), (256, 128), (384, 128)):
                                off = tbi * TB + off
                                po = po_pool.tile(
                                    [128, w], mybir.dt.float32, tag="po"
                                )
                                for si, (kind, k) in enumerate(MM2_SEQ):
                                    if kind == "bf":
                                        nc.tensor.matmul(
                                            po[:],
                                            wd[:, k, :],
                                            hid[:, k, off : off + w],
                                            start=(si == 0),
                                            stop=(si == len(MM2_SEQ) - 1),
                                        )
                                    else:
                                        nc.tensor.matmul(
                                            po[:],
                                            wd8[:, k, :, :],
                                            hid8[:, k, :, off : off + w],
                                            start=False,
                                            stop=False,
                                            perf_mode=mybir.MatmulPerfMode.DoubleRow,
                                        )
                                nc.vector.tensor_copy(
                                    ob[:, off : off + w], po[:]
                                )
                                nc.sync.dma_start(
                                    outT[
                                        ts(hb, 128),
                                        p * TBP * TB + off : p * TBP * TB
                                        + off
                                        + w,
                                    ],
                                    ob[:, off : off + w],
                                )
                            continue
                        po = po_pool.tile([128, TB], mybir.dt.float32, tag="po")
                        for si, (kind, k) in enumerate(MM2_SEQ):
                            if kind == "bf":
                                nc.tensor.matmul(
                                    po[:],
                                    wd[:, k, :],
                                    hid[:, k, ts(tbi, TB)],
                                    start=(si == 0),
                                    stop=(si == len(MM2_SEQ) - 1),
                                )
                            else:
                                nc.tensor.matmul(
                                    po[:],
                                    wd8[:, k, :, :],
                                    hid8[:, k, :, ts(tbi, TB)],
                                    start=False,
                                    stop=False,
                                    perf_mode=mybir.MatmulPerfMode.DoubleRow,
                                )
                        nc.vector.tensor_copy(ob[:, ts(tbi, TB)], po[:])
                        if last:
                            nc.sync.dma_start(
                                outT[ts(hb, 128), ts(p * TBP + tbi, TB)],
                                ob[:, ts(tbi, TB)],
                            )
                    if not last:
                        nc.sync.dma_start(
                            outT[ts(hb, 128), ts(p, TBP * TB)], ob[:]
                        )

    _split_multiwaits(nc, mybir, bass_rust)
    return nc


def _get_nc():
    if "nc" not in _CACHE:
        _CACHE["nc"] = _build()
    return _CACHE["nc"]


def _pack_inputs(x, w_gate_up, w_down):
    """Per-core bf16/e4m3 tile-layout packing (layouts match SBUF exactly)."""
    x = np.asarray(x, dtype=np.float32)
    w_gate_up = np.asarray(w_gate_up, dtype=np.float32)
    w_down = np.asarray(w_down, dtype=np.float32)
    in_maps = []
    for e in range(NCORES):
        xe = x[e * TPE : (e + 1) * TPE].astype(BF16)        # [t, h]
        xt = np.ascontiguousarray(
            xe.reshape(NTB, TB, NHO, 128).transpose(3, 0, 2, 1)
        )                                                    # [hi, tb, ho, t]
        wgu_e = w_gate_up[e].copy()                          # [2F, H] fp32
        # up-projection rows for the fp8 fo-blocks carry the 1/DRS scale so
        # the DVE's e4m3 hid write needs no extra scaling op
        wgu_e[F + NFO_BF * 128 :] *= 1.0 / DRS
        wgu_dev = np.ascontiguousarray(
            wgu_e.astype(BF16)
            .reshape(2, NFO, 128, NHO, 128)
            .transpose(1, 4, 0, 3, 2)
        )                                                    # [jp, hi, half, ho, f]
        wdn_e = w_down[e]                                    # [H, F] fp32
        wdn_t = wdn_e.reshape(NHB, 128, NFO, 128).transpose(0, 3, 2, 1)
        wdn_dev = np.ascontiguousarray(
            wdn_t[:, :, :NFO_BF, :].astype(BF16)
        )                                                    # [b, fi, fo, hh]
        wdn8_dev = np.ascontiguousarray(
            (wdn_t[:, :, NFO_BF:, :] * DRS)
            .reshape(NHB, 128, NPAIR, 2, 128)
            .astype(F8)
        )                                                    # [b, fi, j, i, hh]
        in_maps.append(
            {"xt": xt, "wgu": wgu_dev, "wdn": wdn_dev, "wdn8": wdn8_dev}
        )
    return in_maps


def _fixed_run_bass_via_pjrt(nc, in_maps, n_cores):
    """run_bass_via_pjrt with explicitly device-placed shards.

    The stock version passes host numpy globals into a shard_map'd jit; the
    axon PJRT backend then materializes each device shard via a compiled
    jit_dynamic_slice program, which the stock neuronx-cc takes >25 min to
    compile for our ~270 MB weight arrays.  Building the global arrays from
    per-device buffers (plain H2D copies) avoids any resharding program.
    """
    import jax
    import numpy as np
    from jax.sharding import Mesh, NamedSharding, PartitionSpec
    from jax.experimental.shard_map import shard_map
    import concourse.mybir as mybir
    from concourse import bass2jax

    bass2jax.install_neuronx_cc_hook()
    assert nc.dbg_addr is None
    partition_name = nc.partition_id_tensor.name if nc.partition_id_tensor else None

    in_names, out_names, out_avals, zero_outs = [], [], [], []
    for alloc in nc.m.functions[0].allocations:
        if not isinstance(alloc, mybir.MemoryLocationSet):
            continue
        name = alloc.memorylocations[0].name
        if alloc.kind == "ExternalInput":
            if name != partition_name:
                in_names.append(name)
        elif alloc.kind == "ExternalOutput":
            shape = tuple(alloc.tensor_shape)
            dtype = mybir.dt.np(alloc.dtype)
            out_names.append(name)
            out_avals.append(jax.core.ShapedArray(shape, dtype))
            zero_outs.append(np.zeros(shape, dtype))
    n_params = len(in_names)
    n_outs = len(out_avals)
    in_names.extend(out_names)
    if partition_name is not None:
        in_names.append(partition_name)

    donate = tuple(range(n_params, n_params + n_outs))

    def _body(*args):
        operands = list(args)
        if partition_name is not None:
            operands.append(bass2jax.partition_id_tensor())
        outs = bass2jax._bass_exec_p.bind(
            *operands,
            out_avals=tuple(out_avals),
            in_names=tuple(in_names),
            out_names=tuple(out_names),
            lowering_input_output_aliases=(),
            sim_require_finite=True,
            sim_require_nnan=True,
            nc=nc,
        )
        return tuple(outs)

    devices = jax.devices()[:n_cores]
    mesh = Mesh(np.asarray(devices), ("core",))
    sharding = NamedSharding(mesh, PartitionSpec("core"))
    in_specs = (PartitionSpec("core"),) * (n_params + n_outs)
    out_specs = (PartitionSpec("core"),) * n_outs
    sharded = jax.jit(
        shard_map(
            _body, mesh=mesh, in_specs=in_specs, out_specs=out_specs, check_rep=False
        ),
        donate_argnums=donate,
        keep_unused=True,
    )

    def _make_global(per_core_arrays):
        shape0 = per_core_arrays[0].shape
        gshape = (n_cores * shape0[0], *shape0[1:])
        shards = [
            jax.device_put(per_core_arrays[c], devices[c]) for c in range(n_cores)
        ]
        return jax.make_array_from_single_device_arrays(gshape, sharding, shards)

    global_in = [
        _make_global([np.asarray(m[in_names[i]]) for m in in_maps])
        for i in range(n_params)
    ]
    global_zero = [_make_global([z] * n_cores) for z in zero_outs]

    out_arrs = sharded(*global_in, *global_zero)
    return [
        {
            name: np.asarray(out_arrs[i]).reshape(n_cores, *out_avals[i].shape)[c]
            for i, name in enumerate(out_names)
        }
        for c in range(n_cores)
    ]


def kernel(x, w_gate_up, w_down, tokens_per_expert, _trace=False):
    from concourse import bass2jax
    from concourse.bass_utils import run_bass_kernel_spmd

    bass2jax.run_bass_via_pjrt = _fixed_run_bass_via_pjrt
    nc = _get_nc()
    in_maps = _pack_inputs(x, w_gate_up, w_down)
    res = run_bass_kernel_spmd(
        nc, in_maps, core_ids=list(range(NCORES)), trace=_trace
    )
    _CACHE["last_result"] = res
    out = np.empty((T, H), dtype=np.float32)
    for e in range(NCORES):
        out[e * TPE : (e + 1) * TPE] = res.results[e]["outT"].T
    return out
